# revision 30
# baseline (speedup 1.0000x reference)
"""Causal attention (B=4, S=2048, D=1024, single head) on 8 trn2 NeuronCores. v7.

Sharding: data-parallel over batch (4) x query-split (2) per batch.
  core (b, 0): query rows [0:512] + [1536:2048]; core (b, 1): [512:1536].

Projection folding (host): A = Wq^T Wk and u = bq Wk fold the Q/K projections
into scores = x_q A x_k^T + (u.x_k) bias; V side out = (P x_k) Wv^T / dn + bv
with bv added on host. No collectives.

fp8-DoubleRow strategy (PSUM fp32; DR pair dim = extra contraction rows):
  tq  = x_q A:    A single-fp8 (64x) stationary, x_q single-fp8 (4x) moving,
                  pairs over d-chunk parity -> 4 DR passes per psum; ec pairs
                  interleaved across two psum pools so ldweights overlap.
  scores:         tq8 (8x fp8) moving x xtk (4x fp8) stationary, DR.
  pt  = exp(...): ACT exp -> fp8 (8x) [P, 2, GQ] pair tiles (2 k-slots).
  dn:             DR ones-matmul over the pt pair tiles (replicated [P, GQ]
                  psum; row 0 used - dual-fp8 ldweights reject narrow tiles).
  ut  = x_k^T P:  xkr single-fp8 (4x), DR pairs = 2 k-slots -> 1 pass per
                  pair (per e-chunk); t-OUTER over chunk-halves so xkr2
                  t-blocks are consumed in DMA arrival order.
  fin = U Wv^T:   bf16 (single-fp8 on either operand fails the error budget).

Early-block overlay: rows q<128 have tiny softmax denominators (n_eff ~ 4) and
dominate the max-err metric under fp8 noise, so the (q<128) x (k<128) block is
recomputed cleanly: host precomputes Ak = 32*(A @ x_k[:128]^T); device runs 8
bf16 matmuls overwriting psum cols 0:128 of (g0, slot 0), a separate bf16 exp
tile (fp8 pt cols zeroed), and bf16 ut/dn accumulation for that block.
Measured rel err 0.0045 (v3 baseline: 0.0108; gate 2e-2).

Engine balance (HW-measured: ACT ~1.27us and DVE ~1.0us per [128,512] tile op,
~2x the cost model; Pool cannot read PSUM): ACT runs the exps + half the fin
normalizes (activation scale=inv AP); DVE runs tq fp8 stores, ut drains, the
other fin normalizes, reciprocal; Pool applies multiplicative {0,1} fp8 causal
masks post-exp and issues input/output DMAs (25ns/issue vs 565 on SP).
PE warm-up matmuls hold the p-state through the inter-iteration DMA head.
"""

import numpy as np
import ml_dtypes

import concourse.bass as bass
import concourse.bacc as bacc
import concourse.mybir as mybir
import concourse.tile as tile
from concourse.bass_utils import run_bass_kernel_spmd

BF16 = mybir.dt.bfloat16
FP32 = mybir.dt.float32
FP8 = mybir.dt.float8e4

B, S, D = 4, 2048, 1024
SCALE = 1.0 / np.sqrt(D)
P = 128                  # partition width
DC = D // P              # 8 feature chunks
KB = S // P              # 16 k-blocks
NQ = 1024                # query rows per core
GROUPS = 2               # 512-query groups per core
GQ = 512                 # queries per group
QC = GQ // P             # 4 query chunks of 128 per group
KSLOTS = (8, 16)         # k-block slots per group (SPMD-uniform union)
N_AMASK = 15             # additive mask tiles: g0 s1-7, g1 s8-15
EQ = 128                 # overlay rows (first q-chunk of g0)

_CACHE = {}


def _build_program(reps=1, loop_n=0):
    nc = bacc.Bacc("TRN2", target_bir_lowering=False, debug=False, num_devices=8)

    at8 = nc.dram_tensor("at8", [P, DC * D], FP8, kind="ExternalInput").ap()
    xq8 = nc.dram_tensor("xq8", [P, GROUPS * DC * GQ], FP8,
                         kind="ExternalInput").ap()
    xqb = nc.dram_tensor("xqb", [P, DC * EQ], BF16, kind="ExternalInput").ap()
    akb = nc.dram_tensor("akb", [P, DC * EQ], BF16, kind="ExternalInput").ap()
    xTk = nc.dram_tensor("xTk", [P, DC * S], FP8, kind="ExternalInput").ap()
    xkr2 = nc.dram_tensor("xkr2", [P, KB * D], FP8,
                          kind="ExternalInput").ap()
    xkrb0 = nc.dram_tensor("xkrb0", [P, D], BF16, kind="ExternalInput").ap()
    wvT = nc.dram_tensor("wvT", [P, DC * D], BF16, kind="ExternalInput").ap()
    kb = nc.dram_tensor("kb", [P, KB], FP32, kind="ExternalInput").ap()
    masks = nc.dram_tensor("masks", [P, N_AMASK * GQ], FP8,
                           kind="ExternalInput").ap()
    movl = nc.dram_tensor("movl", [P, EQ], BF16, kind="ExternalInput").ap()
    out = nc.dram_tensor("out", [NQ, D], BF16, kind="ExternalOutput").ap()

    with tile.TileContext(nc) as tc:
        if loop_n:
            with tc.For_i(0, loop_n, 1):
                _emit(tc, at8, xq8, xqb, akb, xTk, xkr2, xkrb0, wvT, kb,
                      masks, movl, out)
        else:
            for _ in range(reps):
                _emit(tc, at8, xq8, xqb, akb, xTk, xkr2, xkrb0, wvT, kb,
                      masks, movl, out)
    nc.compile()
    return nc


def _emit(tc, at8, xq8, xqb, akb, xTk, xkr2, xkrb0, wvT, kb, masks, movl,
          out):
    nc = tc.nc

    with tc.tile_pool(name="persist", bufs=1) as persist:
        at8_sb = persist.tile([P, DC // 2, 2, D], FP8, name="at8_sb")
        xq8_sb = persist.tile([P, GROUPS, DC // 2, 2, GQ], FP8, name="xq8_sb")
        xqb_sb = persist.tile([P, DC, EQ], BF16, name="xqb_sb")
        akb_sb = persist.tile([P, DC, EQ], BF16, name="akb_sb")
        xtk_sb = persist.tile([P, 2, DC // 2, 2, 1024], FP8, name="xtk_sb")
        xkr2_sb = persist.tile([P, KB // 2, 2, D], FP8, name="xkr2_sb")
        xkrb0_sb = persist.tile([P, D], BF16, name="xkrb0_sb")
        wv_sb = persist.tile([P, DC, D], BF16, name="wv_sb")
        tq_sb = persist.tile([P, GROUPS, DC // 2, 2, GQ], FP8, name="tq_sb")
        ut_sb = persist.tile([P, GROUPS, DC, GQ], BF16, name="ut_sb")
        kb_sb = persist.tile([P, KB], FP32, name="kb_sb")
        mk_sb = persist.tile([P, N_AMASK, GQ], FP8, name="mk_sb")
        movl_sb = persist.tile([P, EQ], BF16, name="movl_sb")
        ones8_sb = persist.tile([P, 2, P], FP8, name="ones8_sb")
        onesb_sb = persist.tile([P, 1], BF16, name="onesb_sb")
        onef_sb = persist.tile([P, 1], FP32, name="onef_sb")
        dn_sb = persist.tile([P, GROUPS, GQ], FP32, name="dn_sb")

        warm_sb = persist.tile([P, 2, GQ], FP8, name="warm_sb")
        nc.any.memset(ones8_sb[:], 1.0)
        nc.any.memset(onesb_sb[:], 1.0)
        nc.any.memset(onef_sb[:], 4.0)   # dnp = 4*dn8 = 32*dn -> inv = 1/(32 dn)
        nc.vector.memset(warm_sb[:], 0.0)

        # ---- input DMA: [P,2048]-ish pieces, round-robin over the two idle
        # issue queues (SP, Pool), ordered by first use ----
        qs = [nc.sync, nc.gpsimd]
        qi = [0]

        def load(dst, src_ap, cols=2048):
            n = src_ap.shape[-1]
            for c0 in range(0, n, cols):
                c1 = min(c0 + cols, n)
                qs[qi[0] % 2].dma_start(dst[..., c0:c1], src_ap[..., c0:c1])
                qi[0] += 1

        xtk_flat = xtk_sb.rearrange("p h a o b -> p (h a o b)")
        mk_flat = mk_sb.rearrange("p m b -> p (m b)")
        xkr2_flat = xkr2_sb.rearrange("p t o b -> p (t o b)")

        # tq critical prefix: at8 + xq8 (fp8)
        load(at8_sb.rearrange("p a b c -> p (a b c)"), at8[:])
        load(xq8_sb.rearrange("p g a b c -> p (g a b c)"), xq8[:])
        nc.gpsimd.dma_start(kb_sb[:], kb[:])  # noqa
        # scores slots 1-7: xtk half 0 + masks idx 0-7
        load(xtk_flat[:, 0:DC * 1024], xTk[:, 0:DC * 1024])
        load(mk_flat[:, 0:8 * GQ], masks[:, 0:8 * GQ])
        # scores slots 8-15: xtk half 1 + masks idx 8-14
        load(xtk_flat[:, DC * 1024:], xTk[:, DC * 1024:])
        load(mk_flat[:, 8 * GQ:], masks[:, 8 * GQ:])
        # overlay operands (used after the main score slots)
        load(xqb_sb.rearrange("p a b -> p (a b)"), xqb[:])
        load(akb_sb.rearrange("p a b -> p (a b)"), akb[:])
        nc.gpsimd.dma_start(movl_sb[:], movl[:])
        # ut: xkr2 t-blocks in accumulation order 1..7, 0
        TBLK = 2 * D
        for t in (1, 2, 3, 4, 5, 6, 7, 0):
            load(xkr2_flat[:, t * TBLK:(t + 1) * TBLK],
                 xkr2[:, t * TBLK:(t + 1) * TBLK])
        nc.gpsimd.dma_start(xkrb0_sb[:], xkrb0[:])
        # wv for fin
        load(wv_sb.rearrange("p a b -> p (a b)"), wvT[:])

        # PSUM budget (8 banks): pj 2 + sc 2 + ut 2 + dn 2 = 8.
        with tc.tile_pool(name="pj_psum", bufs=2, space="PSUM") as pj_psum, \
             tc.tile_pool(name="sc_psum", bufs=2, space="PSUM") as sc_psum, \
             tc.tile_pool(name="ut_psum", bufs=2, space="PSUM") as ut_psum, \
             tc.tile_pool(name="dn_psum", bufs=2, space="PSUM") as dn_psum, \
             tc.tile_pool(name="pt", bufs=14) as pt_pool, \
             tc.tile_pool(name="sm", bufs=6) as sm_pool, \
             tc.tile_pool(name="ob", bufs=3) as ob_pool:

            # PE p-state warm-up: ~4us of throwaway DR matmuls while the
            # first input DMAs land, so tq runs at full clock.
            wps = ut_psum.tile([P, GQ], FP32, tag="ut", name="wps")
            for w in range(12):
                nc.tensor.matmul(
                    wps[:], ones8_sb[:], warm_sb[:],
                    start=(w == 0), stop=(w == 11),
                    perf_mode=mybir.MatmulPerfMode.DoubleRow,
                )

            # ---------------- tq = (x_q A)^T: fp8 DR, 4 passes ----------------
            # ec pairs interleaved so consecutive PE instructions alternate
            # psum banks (hides ldweights behind the other psum's matmul)
            for g in range(GROUPS):
                for e0 in range(0, DC, 2):
                    ps0 = pj_psum.tile([P, GQ], FP32, tag="pj", name="ps_tq")
                    ps1 = ut_psum.tile([P, GQ], FP32, tag="ut", name="ps_tq")
                    for j in range(DC // 2):
                        for pi, ec in ((0, e0), (1, e0 + 1)):
                            nc.tensor.matmul(
                                (ps0, ps1)[pi][:],
                                at8_sb[:, j, :, ec * P:(ec + 1) * P],
                                xq8_sb[:, g, j, :, :],
                                start=(j == 0), stop=(j == DC // 2 - 1),
                                perf_mode=mybir.MatmulPerfMode.DoubleRow,
                            )
                    for pi, ec in ((0, e0), (1, e0 + 1)):
                        nc.vector.tensor_scalar_mul(
                            tq_sb[:, g, ec // 2, ec % 2, :], (ps0, ps1)[pi][:],
                            float(1.0 / 32.0),
                        )

            # ------- attention scores + exp: groups interleaved; slot 0 is
            # processed LAST (after the overlay tq_b is ready) -------
            pt_tiles = {}
            ptb_tile = [None]
            sc_ps = {}
            sc_i = [0]

            def emit_slot(s):
                gs = (0, 1) if s < KSLOTS[0] else (1,)
                for g in gs:
                    pool = (sc_psum, pj_psum)[sc_i[0] % 2]
                    tag = ("sc", "pj")[sc_i[0] % 2]
                    sc_i[0] += 1
                    sc_ps[g] = pool.tile([P, GQ], FP32, tag=tag, name="ps_sc")
                for j in range(DC // 2):
                    for g in gs:
                        nc.tensor.matmul(
                            sc_ps[g][:],
                            xtk_sb[:, s // 8, j, :, (s % 8) * P:(s % 8 + 1) * P],
                            tq_sb[:, g, j, :, :],
                            start=(j == 0), stop=(j == DC // 2 - 1),
                            perf_mode=mybir.MatmulPerfMode.DoubleRow,
                        )
                if s == 0:
                    # overlay: overwrite psum cols 0:EQ of (g0, slot 0) with
                    # clean bf16 scores. Stationary Ak = 32*(A @ x_k[:128]^T)
                    # precomputed on host, moving = clean bf16 x_q chunk 0.
                    for dc in range(DC):
                        nc.tensor.matmul(
                            sc_ps[0][:, 0:EQ],
                            akb_sb[:, dc, :],
                            xqb_sb[:, dc, :],
                            start=(dc == 0), stop=(dc == DC - 1),
                        )
                for g in gs:
                    mi = -1
                    if g == 0 and 1 <= s <= 7:
                        mi = s - 1
                    elif g == 1 and s >= 8:
                        mi = 7 + (s - 8)
                    if (g, s // 2) not in pt_tiles:
                        pt_tiles[(g, s // 2)] = pt_pool.tile(
                            [P, 2, GQ], FP8, tag="pt", name="pt", bufs=14)
                    pt = pt_tiles[(g, s // 2)]
                    if g == 0 and s == 0:
                        # fp8 part covers cols EQ: only; cols 0:EQ zeroed, the
                        # bf16 overlay tile carries them instead.
                        nc.vector.memset(pt[:, 0, 0:EQ], 0.0)
                        nc.scalar.activation(
                            pt[:, 0, EQ:], sc_ps[g][:, EQ:],
                            mybir.ActivationFunctionType.Exp,
                            scale=float(SCALE / 32.0), bias=kb_sb[:, s:s + 1],
                        )
                        ptb = sm_pool.tile([P, EQ], BF16, tag="ptb", name="ptb")
                        nc.scalar.activation(
                            ptb[:], sc_ps[g][:, 0:EQ],
                            mybir.ActivationFunctionType.Exp,
                            scale=float(SCALE / 32.0), bias=kb_sb[:, s:s + 1],
                        )
                        nc.vector.tensor_tensor(
                            ptb[:], ptb[:], movl_sb[:],
                            op=mybir.AluOpType.mult,
                        )
                        ptb_tile[0] = ptb
                    else:
                        nc.scalar.activation(
                            pt[:, s % 2, :], sc_ps[g][:],
                            mybir.ActivationFunctionType.Exp,
                            scale=float(SCALE / 32.0), bias=kb_sb[:, s:s + 1],
                        )
                        if mi >= 0:
                            # multiplicative 0/1 fp8 causal mask on the idle
                            # Pool engine (post-exp)
                            nc.gpsimd.tensor_tensor(
                                pt[:, s % 2, :], pt[:, s % 2, :],
                                mk_sb[:, mi, :], op=mybir.AluOpType.mult,
                            )

            for s in range(1, KSLOTS[1]):
                emit_slot(s)
            emit_slot(0)

            # U^T accumulation (fp8 DR hi/lo), t-OUTER over chunk-halves so the
            # xkr2 t-blocks are consumed in DMA arrival order; pair 0 (slots
            # 0/1, incl. overlay) accumulates last. dn / redistribute /
            # reciprocal run between the halves.
            NP = (KSLOTS[0] // 2, KSLOTS[1] // 2)  # pair counts (4, 8)
            T_ORDER = (1, 2, 3, 4, 5, 6, 7, 0)
            POOLS = ((ut_psum, "ut"), (sc_psum, "sc"), (pj_psum, "pj"),
                     (dn_psum, "dn"))
            dn_ps = {}
            dnp = {}
            inv = {}
            for half in range(2):
                cs = list(range(half * 4, half * 4 + 4))
                ut_c = {}
                for ci, c in enumerate(cs):
                    pool, tag = POOLS[ci]
                    for g in range(GROUPS):
                        ut_c[(g, c)] = pool.tile([P, GQ], FP32, tag=tag,
                                                 name="ps_ut")
                for t in T_ORDER:
                    for c in cs:
                        for g in ((0, 1) if t < NP[0] else (1,)):
                            # g0's group is closed by the overlay below
                            nc.tensor.matmul(
                                ut_c[(g, c)][:],
                                xkr2_sb[:, t, :, c * P:(c + 1) * P],
                                pt_tiles[(g, t)][:],
                                start=(t == 1),
                                stop=(g == 1 and t == 0),
                                perf_mode=mybir.MatmulPerfMode.DoubleRow,
                            )
                        if t == 0:
                            # overlay: bf16 x_k(slot0) x ptb into cols 0:EQ
                            nc.tensor.matmul(
                                ut_c[(0, c)][:, 0:EQ],
                                xkrb0_sb[:, c * P:(c + 1) * P],
                                ptb_tile[0][:],
                                start=False, stop=True, skip_group_check=True,
                            )
                # drain psums to SBUF (DVE; ACT is saturated by the exps)
                for ci, c in enumerate(cs):
                    for g in range(GROUPS):
                        nc.vector.tensor_copy(ut_sb[:, g, c, :],
                                              ut_c[(g, c)][:])

                if half == 0:
                    # denominators: dn[1, q] += ones.T @ pt (DR over pair
                    # tiles), then redistribute + reciprocal while half B's
                    # xkr2 still streams in.
                    for g in range(GROUPS):
                        dn_ps[g] = dn_psum.tile([P, GQ], FP32, tag="dn",
                                                name="dn_ps")
                        for t in T_ORDER:
                            if t >= NP[g]:
                                continue
                            # g0's group is closed by the overlay below
                            nc.tensor.matmul(
                                dn_ps[g][:], ones8_sb[:], pt_tiles[(g, t)][:],
                                start=(t == 1),
                                stop=(g == 1 and t == 0),
                                perf_mode=mybir.MatmulPerfMode.DoubleRow,
                            )
                    nc.tensor.matmul(
                        dn_ps[0][0:1, 0:EQ], onesb_sb[:], ptb_tile[0][:],
                        start=False, stop=True, skip_group_check=True,
                    )
                    for g in range(GROUPS):
                        nc.vector.tensor_copy(dn_sb[0:1, g, :], dn_ps[g][0:1, :])
                    for g in range(GROUPS):
                        dnp[g] = dn_psum.tile([P, QC], FP32, tag="dn",
                                              name="dnp")
                        for qc in range(QC):
                            nc.tensor.matmul(
                                dnp[g][:, qc:qc + 1],
                                dn_sb[0:1, g, qc * P:(qc + 1) * P],
                                onef_sb[0:1, 0:1],
                                start=True, stop=True,
                            )
                    for g in range(GROUPS):
                        inv[g] = sm_pool.tile([P, QC], FP32, tag="inv",
                                              name="inv")
                        nc.vector.reciprocal(inv[g][:], dnp[g][:])

            # final projection out[q, :] = (U Wv^T) * inv[q]  (bf16)
            fin_i = [0]
            for g in range(GROUPS):
                for qc in range(QC):
                    row = g * GQ + qc * P
                    for h in range(2):
                        pool = (pj_psum, sc_psum)[fin_i[0] % 2]
                        tag = ("pj", "sc")[fin_i[0] % 2]
                        fin_i[0] += 1
                        pso = pool.tile([P, GQ], FP32, tag=tag, name="ps_o")
                        for c in range(DC):
                            nc.tensor.matmul(
                                pso[:],
                                ut_sb[:, g, c, qc * P:(qc + 1) * P],
                                wv_sb[:, c, h * GQ:(h + 1) * GQ],
                                start=(c == 0), stop=(c == DC - 1),
                            )
                        ob = ob_pool.tile([P, GQ], BF16, tag="ob", name="ob")
                        if fin_i[0] % 2:
                            nc.vector.tensor_scalar_mul(
                                ob[:], pso[:], inv[g][:, qc:qc + 1]
                            )
                        else:
                            nc.scalar.activation(
                                ob[:], pso[:],
                                mybir.ActivationFunctionType.Identity,
                                scale=inv[g][:, qc:qc + 1],
                            )
                        nc.gpsimd.dma_start(
                            out[row:row + P, h * GQ:(h + 1) * GQ], ob[:]
                        )


def _fp8(a):
    return np.clip(a, -240, 240).astype(ml_dtypes.float8_e4m3)


def _chunked_T(a, dtype=ml_dtypes.bfloat16):
    """[rows, D] fp32 -> feature-major [P, DC*rows] (chunk-major free)."""
    rows = a.shape[0]
    t = np.ascontiguousarray(a.T)                      # [D, rows]
    t = t.reshape(DC, P, rows).transpose(1, 0, 2)      # [P, DC, rows]
    return np.ascontiguousarray(t.reshape(P, DC * rows)).astype(dtype)


def _pairs_fp8(a, scale):
    """[rows, D] fp32 -> fp8 pair layout [P, (D/256) * 2 * rows]:
    [p, j(chunk pair), o(parity), r] with d = j*256 + o*128 + p."""
    rows = a.shape[0]
    t = np.clip(a.T * scale, -240, 240)                # [D, rows]
    t = t.reshape(DC // 2, 2, P, rows)                 # [j, o, p, r]
    t = t.transpose(2, 0, 1, 3)                        # [p, j, o, r]
    return np.ascontiguousarray(t.reshape(P, (DC // 2) * 2 * rows)).astype(
        ml_dtypes.float8_e4m3)


def _xtk_fp8(a):
    """[S, D] fp32 -> fp8 x4 pair layout [P, 2*4*2*1024]:
    [p, h(k-half), j(e-chunk pair), o(pair member), k]."""
    t = np.clip(a.T * 4.0, -240, 240)                  # [D, S]
    t = t.reshape(DC // 2, 2, P, 2, 1024)              # [j, o, p, h, k]
    t = t.transpose(2, 3, 0, 1, 4)                     # [p, h, j, o, k]
    return np.ascontiguousarray(t.reshape(P, DC * S)).astype(
        ml_dtypes.float8_e4m3)


def _xkr2_fp8(a):
    """[S, D] fp32 -> single fp8 layout [P, 8*2*D]:
    [p, t(slot pair), o(slot parity), e], value = 4*x[k, e]
    with k = t*256 + o*128 + p."""
    hi = _fp8(a * 4.0)                                 # [S, D]
    t = hi.reshape(KB // 2, 2, P, D)                   # [t, o, p, e]
    t = t.transpose(2, 0, 1, 3)                        # [p, t, o, e]
    return np.ascontiguousarray(t.reshape(P, (KB // 2) * 2 * D))


def _make_amasks(half):
    """Multiplicative fp8 mask tiles [N_AMASK, P, GQ], values {1, 0},
    applied to the fp8 pt tiles post-exp on the Pool engine.

    idx 0-6 -> (g0, slot s=idx+1); idx 7-14 -> (g1, slot s=idx+1)."""
    q_starts = ((0, 1536), (512, 1024))[half]
    m = np.zeros((N_AMASK, P, GQ), np.float32)
    qq = np.arange(GQ)[None, :]
    kk = np.arange(P)[:, None]
    for s in range(1, 8):
        m[s - 1] = (s * P + kk <= q_starts[0] + qq)
    for s in range(8, 16):
        m[7 + s - 8] = (s * P + kk <= q_starts[1] + qq)
    return np.ascontiguousarray(
        m.transpose(1, 0, 2).reshape(P, N_AMASK * GQ)
    ).astype(ml_dtypes.float8_e4m3)


def _host_inputs(x, Wq, bq, Wk, bk, Wv, bv):
    """Build the 8 per-core input maps (and remember bv for assembly)."""
    x = np.asarray(x, np.float32)
    Wq = np.asarray(Wq, np.float32)
    Wk = np.asarray(Wk, np.float32)
    Wv = np.asarray(Wv, np.float32)
    bq = np.asarray(bq, np.float32)

    A = Wq.T @ Wk                     # [D, D] fp32: folds Q and K projections
    u = bq @ Wk                       # [D]: score bias (bq Wk).x_k
    at8 = _pairs_fp8(A.T, 64.0)       # fp8 single, 64x
    wvT = _chunked_T(Wv)
    masks_by_half = [_make_amasks(0), _make_amasks(1)]
    qq = np.arange(EQ)[None, :]
    kk = np.arange(P)[:, None]
    movl_by_half = [
        np.ascontiguousarray((kk <= qq).astype(np.float32)).astype(
            ml_dtypes.bfloat16),
        np.ones((P, EQ), ml_dtypes.bfloat16),
    ]

    in_maps = []
    for core in range(8):
        b, half = core // 2, core % 2
        if half == 0:
            qrows = np.r_[0:512, 1536:2048]
        else:
            qrows = np.r_[512:1536]
        kbias = SCALE * (x[b] @ u) + np.log(8.0)   # pt stored as 8*exp
        xq = x[b][qrows]                           # [NQ, D]
        xq8 = np.concatenate(
            [_pairs_fp8(xq[g * GQ:(g + 1) * GQ], 4.0) for g in range(GROUPS)],
            axis=1)
        Ak = 32.0 * (A @ x[b][0:P].T)              # [D, EQ] fp32, 32x for psum
        in_maps.append({
            "at8": at8,
            "xq8": xq8,
            "xqb": _chunked_T(xq[0:EQ]),
            "akb": _chunked_T(Ak.T),
            "xTk": _xtk_fp8(x[b]),
            "xkr2": _xkr2_fp8(x[b]),
            "xkrb0": np.ascontiguousarray(4.0 * x[b][0:P]).astype(
                ml_dtypes.bfloat16),
            "wvT": wvT,
            "kb": np.ascontiguousarray(kbias.reshape(KB, P).T),
            "masks": masks_by_half[half],
            "movl": movl_by_half[half],
        })
    return in_maps


def kernel(x, Wq, bq, Wk, bk, Wv, bv):
    bv = np.asarray(bv, np.float32)
    in_maps = _host_inputs(x, Wq, bq, Wk, bk, Wv, bv)

    import os
    reps = int(os.environ.get("BENCH_REPS", "1"))
    key = ("nc", reps)
    if key not in _CACHE:
        _CACHE[key] = _build_program(reps)
    res = run_bass_kernel_spmd(_CACHE[key], in_maps, list(range(8)))
    _CACHE["last_results"] = res

    out = np.empty((B, S, D), np.float32)
    for core in range(8):
        o = np.asarray(res.results[core]["out"]).astype(np.float32)
        b, half = core // 2, core % 2
        if half == 0:
            out[b, 0:512] = o[0:512]
            out[b, 1536:2048] = o[512:1024]
        else:
            out[b, 512:1536] = o
    out += bv
    return out
